# revision 4
# baseline (speedup 1.0000x reference)
"""Trainium2 Bass kernel for nn_CFM_80272938762374 (dense_mlp).

Reference computation (T=1024, O=512, D=256, H=512):
    ht = z_t @ W1[:D]                  # [T, H]
    ho = z_o @ W1[D:]                  # [O, H]
    h  = leaky_relu(ht[:,None,:] + ho[None,:,:] + b1, 0.01)   # [T, O, H]
    out = squeeze(h @ W2, -1) + b2[0]  # [T, O]

Strategy (8 cores, O sharded 64-wide per core; all FLOPs on device; host
does only layout prep - transposes, slicing, weight scaling/casts):

    leaky_relu(x) = 0.99*relu(x) + 0.01*x, so with g = ho + b1:
      out[t,o] = sum_k 0.99*W2[k]*relu(htT[k,t] + g[k,o])
               + ct[t] + co[o] + const        # linear terms collapse

    Per core:
    * PE computes htT[k,t] (fp16) and g[k,o] (fp32) once.
    * Main loop (64 o's x 4 k-blocks): one elementwise op produces each
      relu tile r = relu(htT + g[:,o]) [128k x 1024t] fp16; the ops are
      load-balanced across DVE (tensor_scalar, 4x mode), ACT (Relu with
      bias) and GPSIMD (tensor_scalar) so all three engines stream the
      T*O*H/8 relu volume concurrently.
    * The W2 contraction runs with r as the STATIONARY operand: per
      128-wide t-chunk, matmul(lhsT=r[:,tc], rhs=w99[kb] [128,1]) emits a
      [128t, 1] PSUM column. Columns for 16 o's x 8 t-chunks accumulate
      in one PSUM bank (bank zero-filled once via a K=1 zero matmul with
      start=True; all accumulating matmuls use start=False).
    * Linear terms enter the same PSUM bank through cheap N<=16 matmuls:
      ct via lhsT=ztT-chunk / rhs=repeat(0.01*W1a@W2), co+b2 via
      lhsT=ones / rhs=(z_oT * (0.01*W1b@W2) + b2m/256).
    * Output drains per 16-o chunk (DVE copy + DMA), overlapping the
      remaining production; final host step is a concat along O.
"""

import os

os.environ.setdefault("JAX_PLATFORMS", "axon")

import numpy as np

import concourse.bacc as bacc
import concourse.tile as tile
from concourse import mybir
from concourse.bass_utils import run_bass_kernel_spmd

F32 = mybir.dt.float32
FP16 = mybir.dt.float16
AOP = mybir.AluOpType
AF = mybir.ActivationFunctionType

T, O, D, H = 1024, 512, 256, 512
NCORES = 8
OL = O // NCORES          # 64 o's per core
KB = H // 128             # 4 k-blocks
DC = D // 128             # 2 d-chunks
TC = T // 128             # 8 t-chunks
NCH = 4                   # o-chunks per core
OC = OL // NCH            # 16 o's per chunk

_cache = {}


def _prod_schedule():
    """Static assignment of the 256 (o,kb) relu-tile productions to
    engines, greedy-balanced by modeled per-op cost, with fixed setup
    work pre-loaded per engine."""
    cost = {"dve": 327.0, "act": 1038.0, "pool": 853.0}
    load = {"dve": 3860.0, "act": 3400.0, "pool": 0.0}
    sched = []
    for _ in range(OL * KB):
        e = min(cost, key=lambda k: load[k] + cost[k])
        sched.append(e)
        load[e] += cost[e]
    return sched


def _build():
    nc = bacc.Bacc(
        "TRN2", target_bir_lowering=False, debug=False, num_devices=NCORES
    )

    zt_T = nc.dram_tensor("zt_T", [D, T], FP16, kind="ExternalInput").ap()
    w1a = nc.dram_tensor("w1a", [D, H], FP16, kind="ExternalInput").ap()
    w1b = nc.dram_tensor("w1b", [D, H], FP16, kind="ExternalInput").ap()
    # fp16 pack: zo_T (2*OL) | w99 (KB) | v16rep (2*OL)
    pk16 = nc.dram_tensor(
        "pk16", [128, 2 * OL + KB + 2 * OL], FP16, kind="ExternalInput"
    ).ap()
    # f32 pack: b1 (KB) | b2q (1) | cou (DC)
    pk32 = nc.dram_tensor(
        "pk32", [128, KB + 1 + DC], F32, kind="ExternalInput"
    ).ap()
    out_d = nc.dram_tensor("out", [T, OL], F32, kind="ExternalOutput").ap()

    sched = _prod_schedule()

    with tile.TileContext(nc) as tc:
        with (
            tc.tile_pool(name="const", bufs=1) as cpool,
            tc.tile_pool(name="rp_dve", bufs=8) as rp_dve,
            tc.tile_pool(name="rp_act", bufs=6) as rp_act,
            tc.tile_pool(name="rp_gps", bufs=6) as rp_gps,
            tc.tile_pool(name="spool", bufs=4) as spool,
        ):
            # ---- loads (ordered so htT setup can start early) ----
            zt_sb = []
            w1a_sb = []
            for dc in range(DC):
                t_ = cpool.tile([128, T], FP16, name=f"zt{dc}", tag=f"zt{dc}")
                nc.sync.dma_start(out=t_[:], in_=zt_T[dc * 128:(dc + 1) * 128, :])
                zt_sb.append(t_)
                w_ = cpool.tile([128, H], FP16, name=f"w1a{dc}", tag=f"w1a{dc}")
                nc.sync.dma_start(out=w_[:], in_=w1a[dc * 128:(dc + 1) * 128, :])
                w1a_sb.append(w_)
            w1b_sb = []
            for dc in range(DC):
                w_ = cpool.tile([128, H], FP16, name=f"w1b{dc}", tag=f"w1b{dc}")
                nc.sync.dma_start(out=w_[:], in_=w1b[dc * 128:(dc + 1) * 128, :])
                w1b_sb.append(w_)
            p16 = cpool.tile(
                [128, 2 * OL + KB + 2 * OL], FP16, name="p16", tag="p16"
            )
            nc.sync.dma_start(out=p16[:], in_=pk16[:])
            zo_sb = [p16[:, dc * OL:(dc + 1) * OL] for dc in range(DC)]
            w99 = [p16[:, 2 * OL + kb:2 * OL + kb + 1] for kb in range(KB)]
            v16rep = [
                p16[:, 2 * OL + KB + dc * OL:2 * OL + KB + (dc + 1) * OL]
                for dc in range(DC)
            ]
            p32 = cpool.tile([128, KB + 1 + DC], F32, name="p32", tag="p32")
            nc.sync.dma_start(out=p32[:], in_=pk32[:])
            b1_sb = [p32[:, kb:kb + 1] for kb in range(KB)]
            b2q = p32[:, KB:KB + 1]
            cou = [p32[:, KB + 1 + dc:KB + 2 + dc] for dc in range(DC)]

            zcol = cpool.tile([1, 128], FP16, name="zcol", tag="zcol")
            nc.vector.memset(zcol[:], 0.0)
            ones128 = cpool.tile([128, 128], FP16, name="ones128", tag="ones128")
            nc.vector.memset(ones128[:], 1.0)

            # zom[d,o] = zo_T[d,o]*cou[d] + b2m/256 (summed over d -> co+b2)
            zom = cpool.tile([128, 2 * OL], FP16, name="zom", tag="zom")
            for dc in range(DC):
                nc.vector.tensor_scalar(
                    out=zom[:, dc * OL:(dc + 1) * OL], in0=zo_sb[dc],
                    scalar1=cou[dc], scalar2=b2q,
                    op0=AOP.mult, op1=AOP.add,
                )

            # ---- setup: g then htT (PE) ----
            htT = [
                cpool.tile([128, T], FP16, name=f"htT{kb}", tag=f"htT{kb}")
                for kb in range(KB)
            ]
            g_sb = [
                cpool.tile([128, OL], F32, name=f"g{kb}", tag=f"g{kb}")
                for kb in range(KB)
            ]
            cp_eng = [nc.vector, nc.scalar, nc.vector, nc.scalar,
                      nc.vector, nc.scalar, nc.vector, nc.scalar]
            with tc.psum_pool(name="ps_setup", bufs=3) as ps_setup:
                for kb in range(KB):
                    ks = slice(kb * 128, (kb + 1) * 128)
                    pg = ps_setup.tile([128, OL], F32, name="pg", tag="pg")
                    for dc in range(DC):
                        nc.tensor.matmul(
                            pg[:], lhsT=w1b_sb[dc][:, ks], rhs=zo_sb[dc],
                            start=(dc == 0), stop=(dc == DC - 1),
                        )
                    nc.scalar.activation(
                        g_sb[kb][:], pg[:], AF.Identity, bias=b1_sb[kb]
                    )
                for kb in range(KB):
                    ks = slice(kb * 128, (kb + 1) * 128)
                    for th in range(2):
                        ts = slice(th * 512, (th + 1) * 512)
                        pht = ps_setup.tile([128, 512], F32, name="pht", tag="pht")
                        for dc in range(DC):
                            nc.tensor.matmul(
                                pht[:], lhsT=w1a_sb[dc][:, ks],
                                rhs=zt_sb[dc][:, ts],
                                start=(dc == 0), stop=(dc == DC - 1),
                            )
                        eng = cp_eng[kb * 2 + th]
                        if eng is nc.scalar:
                            nc.scalar.activation(htT[kb][:, ts], pht[:], AF.Copy)
                        else:
                            eng.tensor_copy(out=htT[kb][:, ts], in_=pht[:])

            # ---- main loop: 4 chunks of 16 o's ----
            ps_main_ctx = tc.psum_pool(name="ps_main", bufs=3)
            ps_main = ps_main_ctx.__enter__()
            rp = {"dve": rp_dve, "act": rp_act, "pool": rp_gps}
            pi = 0
            for ch in range(NCH):
                o0 = ch * OC
                # full-bank psum tile; only cols [0, TC*OC) used
                P = ps_main.tile([128, 512], F32, name="P", tag="P")
                nc.tensor.matmul(
                    P[:, 0:TC * OC], lhsT=zcol[:], rhs=ones128[0:1, 0:TC * OC],
                    start=True, stop=False, skip_group_check=True,
                )
                # linear terms: ct (via ztT-stationary) and co+b2 (via ones)
                for tcx in range(TC):
                    tsl = slice(tcx * 128, (tcx + 1) * 128)
                    for dc in range(DC):
                        nc.tensor.matmul(
                            P[:, tcx * OC:(tcx + 1) * OC],
                            lhsT=zt_sb[dc][:, tsl],
                            rhs=v16rep[dc][:, 0:OC],
                            start=False, stop=False, skip_group_check=True,
                        )
                        nc.tensor.matmul(
                            P[:, tcx * OC:(tcx + 1) * OC],
                            lhsT=ones128[:],
                            rhs=zom[:, dc * OL + o0:dc * OL + o0 + OC],
                            start=False, stop=False, skip_group_check=True,
                        )
                for oj in range(OC):
                    o = o0 + oj
                    for kb in range(KB):
                        e = sched[pi]
                        pi += 1
                        r = rp[e].tile([128, T], FP16, name="r", tag=f"r_{e}")
                        gcol = g_sb[kb][:, o:o + 1]
                        if e == "act":
                            nc.scalar.activation(
                                r[:], htT[kb][:], AF.Relu, bias=gcol
                            )
                        elif e == "dve":
                            nc.vector.tensor_scalar(
                                out=r[:], in0=htT[kb][:], scalar1=gcol,
                                scalar2=0.0, op0=AOP.add, op1=AOP.max,
                            )
                        else:
                            nc.gpsimd.tensor_scalar(
                                out=r[:], in0=htT[kb][:], scalar1=gcol,
                                scalar2=0.0, op0=AOP.add, op1=AOP.max,
                            )
                        for tcx in range(TC):
                            nc.tensor.matmul(
                                P[:, tcx * OC + oj:tcx * OC + oj + 1],
                                lhsT=r[:, tcx * 128:(tcx + 1) * 128],
                                rhs=w99[kb],
                                start=False, stop=(kb == KB - 1),
                                skip_group_check=True,
                            )
                # drain chunk: one DVE copy + per-t-chunk DMA
                fin = spool.tile([128, TC * OC], F32, name="fin", tag="fin")
                nc.vector.tensor_copy(out=fin[:], in_=P[:, 0:TC * OC])
                for tcx in range(TC):
                    nc.sync.dma_start(
                        out=out_d[tcx * 128:(tcx + 1) * 128, o0:o0 + OC],
                        in_=fin[:, tcx * OC:(tcx + 1) * OC],
                    )
            ps_main_ctx.__exit__(None, None, None)

    nc.compile()
    return nc


def _get_nc():
    if "nc" not in _cache:
        _cache["nc"] = _build()
    return _cache["nc"]


def _host_prep(z_t, z_o, W1, b1, W2, b2):
    """Weight/layout-only host prep; returns per-core input maps."""
    zt_T = np.ascontiguousarray(z_t.T.astype(np.float16))      # [D, T]
    w1a_h = np.ascontiguousarray(W1[:D].astype(np.float16))    # [D, H]
    w1b_h = np.ascontiguousarray(W1[D:].astype(np.float16))    # [D, H]
    w99 = (0.99 * W2).astype(np.float16)                       # [H, 1]
    v = 0.01 * (W1[:D] @ W2)                                   # [D, 1]
    u = 0.01 * (W1[D:] @ W2)                                   # [D, 1]
    b2m = float(b2[0] + 0.01 * (W2[:, 0] @ b1))
    v16rep = np.repeat(v.astype(np.float16), OL, 1)            # [D, OL]
    v16rep = v16rep.reshape(DC, 128, OL).transpose(1, 0, 2).reshape(128, 2 * OL)
    w99p = w99.reshape(KB, 128).T.reshape(128, KB)             # [128, KB]
    cou_p = u.astype(np.float32).reshape(DC, 128).T.reshape(128, DC)
    b1p = b1.reshape(KB, 128).T.reshape(128, KB).astype(np.float32)
    b2qcol = np.full((128, 1), b2m / D, np.float32)
    pk32 = np.ascontiguousarray(np.concatenate([b1p, b2qcol, cou_p], 1))

    in_maps = []
    for c in range(NCORES):
        zo_T = (
            z_o[c * OL:(c + 1) * OL].T.astype(np.float16)
            .reshape(DC, 128, OL).transpose(1, 0, 2).reshape(128, 2 * OL)
        )
        pk16 = np.ascontiguousarray(
            np.concatenate([zo_T, w99p, v16rep], 1)
        )
        in_maps.append({
            "zt_T": zt_T, "w1a": w1a_h, "w1b": w1b_h,
            "pk16": pk16, "pk32": pk32,
        })
    return in_maps


def kernel(z_t, z_o, W1, b1, W2, b2, **run_kwargs):
    z_t = np.asarray(z_t, np.float32)
    z_o = np.asarray(z_o, np.float32)
    W1 = np.asarray(W1, np.float32)
    b1 = np.asarray(b1, np.float32)
    W2 = np.asarray(W2, np.float32)
    b2 = np.asarray(b2, np.float32)

    nc = _get_nc()
    in_maps = _host_prep(z_t, z_o, W1, b1, W2, b2)
    res = run_bass_kernel_spmd(
        nc, in_maps, core_ids=list(range(NCORES)), **run_kwargs
    )
    out = np.concatenate(
        [res.results[c]["out"] for c in range(NCORES)], axis=1
    )  # [T, O]
    if run_kwargs:
        _cache["last_results"] = res
    return np.ascontiguousarray(out).astype(np.float32)


# revision 5
# speedup vs baseline: 1.0315x; 1.0315x over previous
"""Trainium2 Bass kernel for nn_CFM_80272938762374 (dense_mlp).

Reference computation (T=1024, O=512, D=256, H=512):
    ht = z_t @ W1[:D]                  # [T, H]
    ho = z_o @ W1[D:]                  # [O, H]
    h  = leaky_relu(ht[:,None,:] + ho[None,:,:] + b1, 0.01)   # [T, O, H]
    out = squeeze(h @ W2, -1) + b2[0]  # [T, O]

Strategy (8 cores, O sharded 64-wide per core; all FLOPs on device; host
does only layout prep - transposes, slicing, weight scaling/casts):

    leaky_relu(x) = 0.99*relu(x) + 0.01*x, so with g = ho + b1:
      out[t,o] = sum_k 0.99*W2[k]*relu(htT[k,t] + g[k,o])
               + ct[t] + co[o] + const        # linear terms collapse

    Per core:
    * PE computes htT[k,t] (fp16) and g[k,o] (fp32) once.
    * Main loop (64 o's x 4 k-blocks): one elementwise op produces each
      relu tile r = relu(htT + g[:,o]) [128k x 1024t] fp16; the ops are
      load-balanced across DVE (tensor_scalar, 4x mode), ACT (Relu with
      bias) and GPSIMD (tensor_scalar) so all three engines stream the
      T*O*H/8 relu volume concurrently.
    * The W2 contraction runs with r as the STATIONARY operand: per
      128-wide t-chunk, matmul(lhsT=r[:,tc], rhs=w99[kb] [128,1]) emits a
      [128t, 1] PSUM column. Columns for 16 o's x 8 t-chunks accumulate
      in one PSUM bank (bank zero-filled once via a K=1 zero matmul with
      start=True; all accumulating matmuls use start=False).
    * Linear terms enter the same PSUM bank through cheap N<=16 matmuls:
      ct via lhsT=ztT-chunk / rhs=repeat(0.01*W1a@W2), co+b2 via
      lhsT=ones / rhs=(z_oT * (0.01*W1b@W2) + b2m/256).
    * Output drains per 16-o chunk (DVE copy + DMA), overlapping the
      remaining production; final host step is a concat along O.
"""

import os

os.environ.setdefault("JAX_PLATFORMS", "axon")

import numpy as np

import concourse.bacc as bacc
import concourse.tile as tile
from concourse import mybir
from concourse.bass_utils import run_bass_kernel_spmd

F32 = mybir.dt.float32
FP16 = mybir.dt.float16
AOP = mybir.AluOpType
AF = mybir.ActivationFunctionType

T, O, D, H = 1024, 512, 256, 512
NCORES = 8
OL = O // NCORES          # 64 o's per core
KB = H // 128             # 4 k-blocks
DC = D // 128             # 2 d-chunks
TC = T // 128             # 8 t-chunks
NCH = 4                   # o-chunks per core
OC = OL // NCH            # 16 o's per chunk

_cache = {}


def _prod_schedule():
    """Static assignment of the 256 (o,kb) relu-tile productions to
    engines, greedy-balanced by modeled per-op cost, with fixed setup
    work pre-loaded per engine."""
    cost = {"dve": 327.0, "act": 1038.0, "pool": 853.0}
    load = {"dve": 3860.0, "act": 3400.0, "pool": 0.0}
    sched = []
    for _ in range(OL * KB):
        e = min(cost, key=lambda k: load[k] + cost[k])
        sched.append(e)
        load[e] += cost[e]
    for i in range(OL * KB - 4, OL * KB):
        if sched[i] != "dve":
            j = next(j for j in range(OL * KB - 16)
                     if sched[j] == "dve")
            sched[j], sched[i] = sched[i], "dve"
    return sched


def _build():
    nc = bacc.Bacc(
        "TRN2", target_bir_lowering=False, debug=False, num_devices=NCORES
    )

    zt_T = nc.dram_tensor("zt_T", [D, T], FP16, kind="ExternalInput").ap()
    w1a = nc.dram_tensor("w1a", [D, H], FP16, kind="ExternalInput").ap()
    w1b = nc.dram_tensor("w1b", [D, H], FP16, kind="ExternalInput").ap()
    # fp16 pack: zo_T (2*OL) | w99 (KB) | v16rep (2*OL)
    pk16 = nc.dram_tensor(
        "pk16", [128, 2 * OL + KB + 2 * OL], FP16, kind="ExternalInput"
    ).ap()
    # f32 pack: b1 (KB) | b2q (1) | cou (DC)
    pk32 = nc.dram_tensor(
        "pk32", [128, KB + 1 + DC], F32, kind="ExternalInput"
    ).ap()
    out_d = nc.dram_tensor("out", [T, OL], F32, kind="ExternalOutput").ap()

    sched = _prod_schedule()

    with tile.TileContext(nc) as tc:
        with (
            tc.tile_pool(name="const", bufs=1) as cpool,
            tc.tile_pool(name="rp_dve", bufs=8) as rp_dve,
            tc.tile_pool(name="rp_act", bufs=6) as rp_act,
            tc.tile_pool(name="rp_gps", bufs=6) as rp_gps,
            tc.tile_pool(name="spool", bufs=4) as spool,
        ):
            # ---- loads (ordered so htT setup can start early) ----
            zt_sb = []
            w1a_sb = []
            for dc in range(DC):
                t_ = cpool.tile([128, T], FP16, name=f"zt{dc}", tag=f"zt{dc}")
                nc.sync.dma_start(out=t_[:], in_=zt_T[dc * 128:(dc + 1) * 128, :])
                zt_sb.append(t_)
                w_ = cpool.tile([128, H], FP16, name=f"w1a{dc}", tag=f"w1a{dc}")
                nc.sync.dma_start(out=w_[:], in_=w1a[dc * 128:(dc + 1) * 128, :])
                w1a_sb.append(w_)
            w1b_sb = []
            for dc in range(DC):
                w_ = cpool.tile([128, H], FP16, name=f"w1b{dc}", tag=f"w1b{dc}")
                nc.scalar.dma_start(out=w_[:], in_=w1b[dc * 128:(dc + 1) * 128, :])
                w1b_sb.append(w_)
            p16 = cpool.tile(
                [128, 2 * OL + KB + 2 * OL], FP16, name="p16", tag="p16"
            )
            nc.scalar.dma_start(out=p16[:], in_=pk16[:])
            zo_sb = [p16[:, dc * OL:(dc + 1) * OL] for dc in range(DC)]
            w99 = [p16[:, 2 * OL + kb:2 * OL + kb + 1] for kb in range(KB)]
            v16rep = [
                p16[:, 2 * OL + KB + dc * OL:2 * OL + KB + (dc + 1) * OL]
                for dc in range(DC)
            ]
            p32 = cpool.tile([128, KB + 1 + DC], F32, name="p32", tag="p32")
            nc.scalar.dma_start(out=p32[:], in_=pk32[:])
            b1_sb = [p32[:, kb:kb + 1] for kb in range(KB)]
            b2q = p32[:, KB:KB + 1]
            cou = [p32[:, KB + 1 + dc:KB + 2 + dc] for dc in range(DC)]

            zcol = cpool.tile([1, 128], FP16, name="zcol", tag="zcol")
            nc.vector.memset(zcol[:], 0.0)
            ones128 = cpool.tile([128, 128], FP16, name="ones128", tag="ones128")
            nc.vector.memset(ones128[:], 1.0)

            # zom[d,o] = zo_T[d,o]*cou[d] + b2m/256 (summed over d -> co+b2)
            zom = cpool.tile([128, 2 * OL], FP16, name="zom", tag="zom")
            for dc in range(DC):
                nc.vector.tensor_scalar(
                    out=zom[:, dc * OL:(dc + 1) * OL], in0=zo_sb[dc],
                    scalar1=cou[dc], scalar2=b2q,
                    op0=AOP.mult, op1=AOP.add,
                )

            # ---- setup: g then htT (PE) ----
            htT = [
                cpool.tile([128, T], FP16, name=f"htT{kb}", tag=f"htT{kb}")
                for kb in range(KB)
            ]
            g_sb = [
                cpool.tile([128, OL], F32, name=f"g{kb}", tag=f"g{kb}")
                for kb in range(KB)
            ]
            cp_eng = [nc.vector, nc.scalar, nc.vector, nc.scalar,
                      nc.vector, nc.scalar, nc.vector, nc.scalar]
            with tc.psum_pool(name="ps_setup", bufs=3) as ps_setup:
                for kb in range(KB):
                    ks = slice(kb * 128, (kb + 1) * 128)
                    pg = ps_setup.tile([128, OL], F32, name="pg", tag="pg")
                    for dc in range(DC):
                        nc.tensor.matmul(
                            pg[:], lhsT=w1b_sb[dc][:, ks], rhs=zo_sb[dc],
                            start=(dc == 0), stop=(dc == DC - 1),
                        )
                    nc.scalar.activation(
                        g_sb[kb][:], pg[:], AF.Identity, bias=b1_sb[kb]
                    )
                for kb in range(KB):
                    ks = slice(kb * 128, (kb + 1) * 128)
                    for th in range(2):
                        ts = slice(th * 512, (th + 1) * 512)
                        pht = ps_setup.tile([128, 512], F32, name="pht", tag="pht")
                        for dc in range(DC):
                            nc.tensor.matmul(
                                pht[:], lhsT=w1a_sb[dc][:, ks],
                                rhs=zt_sb[dc][:, ts],
                                start=(dc == 0), stop=(dc == DC - 1),
                            )
                        eng = cp_eng[kb * 2 + th]
                        if eng is nc.scalar:
                            nc.scalar.activation(htT[kb][:, ts], pht[:], AF.Copy)
                        else:
                            eng.tensor_copy(out=htT[kb][:, ts], in_=pht[:])

            # ---- main loop: 4 chunks of 16 o's ----
            ps_main_ctx = tc.psum_pool(name="ps_main", bufs=4)
            ps_main = ps_main_ctx.__enter__()
            rp = {"dve": rp_dve, "act": rp_act, "pool": rp_gps}
            pi = 0
            for ch in range(NCH):
                o0 = ch * OC
                # full-bank psum tile; only cols [0, TC*OC) used
                P = ps_main.tile([128, 512], F32, name="P", tag="P")
                nc.tensor.matmul(
                    P[:, 0:TC * OC], lhsT=zcol[:], rhs=ones128[0:1, 0:TC * OC],
                    start=True, stop=False, skip_group_check=True,
                )
                # linear terms: ct (via ztT-stationary) and co+b2 (via ones)
                for tcx in range(TC):
                    tsl = slice(tcx * 128, (tcx + 1) * 128)
                    for dc in range(DC):
                        nc.tensor.matmul(
                            P[:, tcx * OC:(tcx + 1) * OC],
                            lhsT=zt_sb[dc][:, tsl],
                            rhs=v16rep[dc][:, 0:OC],
                            start=False, stop=False, skip_group_check=True,
                        )
                        nc.tensor.matmul(
                            P[:, tcx * OC:(tcx + 1) * OC],
                            lhsT=ones128[:],
                            rhs=zom[:, dc * OL + o0:dc * OL + o0 + OC],
                            start=False, stop=False, skip_group_check=True,
                        )
                for oj in range(OC):
                    o = o0 + oj
                    for kb in range(KB):
                        e = sched[pi]
                        pi += 1
                        r = rp[e].tile([128, T], FP16, name="r", tag=f"r_{e}")
                        gcol = g_sb[kb][:, o:o + 1]
                        if e == "act":
                            nc.scalar.activation(
                                r[:], htT[kb][:], AF.Relu, bias=gcol
                            )
                        elif e == "dve":
                            nc.vector.tensor_scalar(
                                out=r[:], in0=htT[kb][:], scalar1=gcol,
                                scalar2=0.0, op0=AOP.add, op1=AOP.max,
                            )
                        else:
                            nc.gpsimd.tensor_scalar(
                                out=r[:], in0=htT[kb][:], scalar1=gcol,
                                scalar2=0.0, op0=AOP.add, op1=AOP.max,
                            )
                        for tcx in range(TC):
                            nc.tensor.matmul(
                                P[:, tcx * OC + oj:tcx * OC + oj + 1],
                                lhsT=r[:, tcx * 128:(tcx + 1) * 128],
                                rhs=w99[kb],
                                start=False, stop=(kb == KB - 1),
                                skip_group_check=True,
                            )
                # drain chunk: one DVE copy + per-t-chunk DMA
                fin = spool.tile([128, TC * OC], F32, name="fin", tag="fin")
                nc.vector.tensor_copy(out=fin[:], in_=P[:, 0:TC * OC])
                nc.sync.dma_start(
                    out=out_d[:, o0:o0 + OC].rearrange(
                        "(t p) o -> p t o", p=128
                    ),
                    in_=fin[:],
                )
            ps_main_ctx.__exit__(None, None, None)

    nc.compile()
    return nc


def _get_nc():
    if "nc" not in _cache:
        _cache["nc"] = _build()
    return _cache["nc"]


def _host_prep(z_t, z_o, W1, b1, W2, b2):
    """Weight/layout-only host prep; returns per-core input maps."""
    zt_T = np.ascontiguousarray(z_t.T.astype(np.float16))      # [D, T]
    w1a_h = np.ascontiguousarray(W1[:D].astype(np.float16))    # [D, H]
    w1b_h = np.ascontiguousarray(W1[D:].astype(np.float16))    # [D, H]
    w99 = (0.99 * W2).astype(np.float16)                       # [H, 1]
    v = 0.01 * (W1[:D] @ W2)                                   # [D, 1]
    u = 0.01 * (W1[D:] @ W2)                                   # [D, 1]
    b2m = float(b2[0] + 0.01 * (W2[:, 0] @ b1))
    v16rep = np.repeat(v.astype(np.float16), OL, 1)            # [D, OL]
    v16rep = v16rep.reshape(DC, 128, OL).transpose(1, 0, 2).reshape(128, 2 * OL)
    w99p = w99.reshape(KB, 128).T.reshape(128, KB)             # [128, KB]
    cou_p = u.astype(np.float32).reshape(DC, 128).T.reshape(128, DC)
    b1p = b1.reshape(KB, 128).T.reshape(128, KB).astype(np.float32)
    b2qcol = np.full((128, 1), b2m / D, np.float32)
    pk32 = np.ascontiguousarray(np.concatenate([b1p, b2qcol, cou_p], 1))

    in_maps = []
    for c in range(NCORES):
        zo_T = (
            z_o[c * OL:(c + 1) * OL].T.astype(np.float16)
            .reshape(DC, 128, OL).transpose(1, 0, 2).reshape(128, 2 * OL)
        )
        pk16 = np.ascontiguousarray(
            np.concatenate([zo_T, w99p, v16rep], 1)
        )
        in_maps.append({
            "zt_T": zt_T, "w1a": w1a_h, "w1b": w1b_h,
            "pk16": pk16, "pk32": pk32,
        })
    return in_maps


def kernel(z_t, z_o, W1, b1, W2, b2, **run_kwargs):
    z_t = np.asarray(z_t, np.float32)
    z_o = np.asarray(z_o, np.float32)
    W1 = np.asarray(W1, np.float32)
    b1 = np.asarray(b1, np.float32)
    W2 = np.asarray(W2, np.float32)
    b2 = np.asarray(b2, np.float32)

    nc = _get_nc()
    in_maps = _host_prep(z_t, z_o, W1, b1, W2, b2)
    res = run_bass_kernel_spmd(
        nc, in_maps, core_ids=list(range(NCORES)), **run_kwargs
    )
    out = np.concatenate(
        [res.results[c]["out"] for c in range(NCORES)], axis=1
    )  # [T, O]
    if run_kwargs:
        _cache["last_results"] = res
    return np.ascontiguousarray(out).astype(np.float32)


# revision 6
# speedup vs baseline: 1.0427x; 1.0108x over previous
"""Trainium2 Bass kernel for nn_CFM_80272938762374 (dense_mlp).

Reference computation (T=1024, O=512, D=256, H=512):
    ht = z_t @ W1[:D]                  # [T, H]
    ho = z_o @ W1[D:]                  # [O, H]
    h  = leaky_relu(ht[:,None,:] + ho[None,:,:] + b1, 0.01)   # [T, O, H]
    out = squeeze(h @ W2, -1) + b2[0]  # [T, O]

Strategy (8 cores, O sharded 64-wide per core; all FLOPs on device; host
does only layout prep - transposes, slicing, weight scaling/casts):

    leaky_relu(x) = 0.99*relu(x) + 0.01*x, so with g = ho + b1:
      out[t,o] = sum_k 0.99*W2[k]*relu(htT[k,t] + g[k,o])
               + ct[t] + co[o] + const        # linear terms collapse

    Per core:
    * PE computes htT[k,t] (fp16) and g[k,o] (fp32) once.
    * Main loop (64 o's x 4 k-blocks): one elementwise op produces each
      relu tile r = relu(htT + g[:,o]) [128k x 1024t] fp16; the ops are
      load-balanced across DVE (tensor_scalar, 4x mode), ACT (Relu with
      bias) and GPSIMD (tensor_scalar) so all three engines stream the
      T*O*H/8 relu volume concurrently.
    * The W2 contraction runs with r as the STATIONARY operand: per
      128-wide t-chunk, matmul(lhsT=r[:,tc], rhs=w99[kb] [128,1]) emits a
      [128t, 1] PSUM column. Columns for 16 o's x 8 t-chunks accumulate
      in one PSUM bank (bank zero-filled once via a K=1 zero matmul with
      start=True; all accumulating matmuls use start=False).
    * Linear terms enter the same PSUM bank through cheap N<=16 matmuls:
      ct via lhsT=ztT-chunk / rhs=repeat(0.01*W1a@W2), co+b2 via
      lhsT=ones / rhs=(z_oT * (0.01*W1b@W2) + b2m/256).
    * Output drains per 16-o chunk (DVE copy + DMA), overlapping the
      remaining production; final host step is a concat along O.
"""

import os

os.environ.setdefault("JAX_PLATFORMS", "axon")

import numpy as np

import concourse.bacc as bacc
import concourse.tile as tile
from concourse import mybir
from concourse.bass_utils import run_bass_kernel_spmd

F32 = mybir.dt.float32
FP16 = mybir.dt.float16
AOP = mybir.AluOpType
AF = mybir.ActivationFunctionType

T, O, D, H = 1024, 512, 256, 512
NCORES = 8
OL = O // NCORES          # 64 o's per core
KB = H // 128             # 4 k-blocks
DC = D // 128             # 2 d-chunks
TC = T // 128             # 8 t-chunks
NCH = 4                   # o-chunks per core
OC = OL // NCH            # 16 o's per chunk

_cache = {}


def _prod_schedule():
    """Static assignment of the 256 (o,kb) relu-tile productions to
    engines, greedy-balanced by modeled per-op cost, with fixed setup
    work pre-loaded per engine."""
    cost = {"dve": 327.0, "act": 1038.0, "pool": 853.0}
    load = {"dve": 3700.0, "act": 4700.0, "pool": 0.0}
    sched = []
    for _ in range(OL * KB):
        e = min(cost, key=lambda k: load[k] + cost[k])
        sched.append(e)
        load[e] += cost[e]
    for i in range(OL * KB - 4, OL * KB):
        if sched[i] != "dve":
            j = next(j for j in range(OL * KB - 16)
                     if sched[j] == "dve")
            sched[j], sched[i] = sched[i], "dve"
    return sched


def _build():
    nc = bacc.Bacc(
        "TRN2", target_bir_lowering=False, debug=False, num_devices=NCORES
    )

    zt_T = nc.dram_tensor("zt_T", [D, T], FP16, kind="ExternalInput").ap()
    w1a = nc.dram_tensor("w1a", [D, H], FP16, kind="ExternalInput").ap()
    w1b = nc.dram_tensor("w1b", [D, H], FP16, kind="ExternalInput").ap()
    # fp16 pack: zo_T (2*OL) | w99 (KB) | v16rep (2*OL)
    pk16 = nc.dram_tensor(
        "pk16", [128, 2 * OL + KB + 2 * OL], FP16, kind="ExternalInput"
    ).ap()
    # f32 pack: b1 (KB) | b2q (1) | cou (DC)
    pk32 = nc.dram_tensor(
        "pk32", [128, KB + 1 + DC], F32, kind="ExternalInput"
    ).ap()
    # per-chunk contiguous: [p, ch*128 + tc*OC + o]; host de-interleaves
    out_d = nc.dram_tensor(
        "out", [128, NCH * TC * OC], F32, kind="ExternalOutput"
    ).ap()

    sched = _prod_schedule()

    with tile.TileContext(nc) as tc:
        with (
            tc.tile_pool(name="const", bufs=1) as cpool,
            tc.tile_pool(name="rp_dve", bufs=12) as rp_dve,
            tc.tile_pool(name="rp_act", bufs=8) as rp_act,
            tc.tile_pool(name="rp_gps", bufs=8) as rp_gps,
            tc.tile_pool(name="spool", bufs=4) as spool,
        ):
            # ---- loads (ordered so htT setup can start early) ----
            zt_sb = []
            w1a_sb = []
            for dc in range(DC):
                t_ = cpool.tile([128, T], FP16, name=f"zt{dc}", tag=f"zt{dc}")
                nc.sync.dma_start(out=t_[:], in_=zt_T[dc * 128:(dc + 1) * 128, :])
                zt_sb.append(t_)
                w_ = cpool.tile([128, H], FP16, name=f"w1a{dc}", tag=f"w1a{dc}")
                nc.sync.dma_start(out=w_[:], in_=w1a[dc * 128:(dc + 1) * 128, :])
                w1a_sb.append(w_)
            w1b_sb = []
            for dc in range(DC):
                w_ = cpool.tile([128, H], FP16, name=f"w1b{dc}", tag=f"w1b{dc}")
                nc.scalar.dma_start(out=w_[:], in_=w1b[dc * 128:(dc + 1) * 128, :])
                w1b_sb.append(w_)
            p16 = cpool.tile(
                [128, 2 * OL + KB + 2 * OL], FP16, name="p16", tag="p16"
            )
            nc.scalar.dma_start(out=p16[:], in_=pk16[:])
            zo_sb = [p16[:, dc * OL:(dc + 1) * OL] for dc in range(DC)]
            w99 = [p16[:, 2 * OL + kb:2 * OL + kb + 1] for kb in range(KB)]
            v16rep = [
                p16[:, 2 * OL + KB + dc * OL:2 * OL + KB + (dc + 1) * OL]
                for dc in range(DC)
            ]
            p32 = cpool.tile([128, KB + 1 + DC], F32, name="p32", tag="p32")
            nc.scalar.dma_start(out=p32[:], in_=pk32[:])
            b1_sb = [p32[:, kb:kb + 1] for kb in range(KB)]
            b2q = p32[:, KB:KB + 1]
            cou = [p32[:, KB + 1 + dc:KB + 2 + dc] for dc in range(DC)]

            zcol = cpool.tile([1, 128], FP16, name="zcol", tag="zcol")
            nc.vector.memset(zcol[:], 0.0)
            ones128 = cpool.tile([128, 128], FP16, name="ones128", tag="ones128")
            nc.vector.memset(ones128[:], 1.0)

            # zom[d,o] = zo_T[d,o]*cou[d] + b2m/256 (summed over d -> co+b2)
            zom = cpool.tile([128, 2 * OL], FP16, name="zom", tag="zom")
            for dc in range(DC):
                nc.vector.tensor_scalar(
                    out=zom[:, dc * OL:(dc + 1) * OL], in0=zo_sb[dc],
                    scalar1=cou[dc], scalar2=b2q,
                    op0=AOP.mult, op1=AOP.add,
                )

            # ---- setup: g then htT (PE) ----
            htT = [
                cpool.tile([128, T], FP16, name=f"htT{kb}", tag=f"htT{kb}")
                for kb in range(KB)
            ]
            g_sb = [
                cpool.tile([128, OL], F32, name=f"g{kb}", tag=f"g{kb}")
                for kb in range(KB)
            ]
            cp_eng = [nc.vector, nc.scalar, nc.vector, nc.scalar,
                      nc.vector, nc.scalar, nc.vector, nc.scalar]
            with tc.psum_pool(name="ps_setup", bufs=3) as ps_setup:
                for kb in range(KB):
                    ks = slice(kb * 128, (kb + 1) * 128)
                    pg = ps_setup.tile([128, OL], F32, name="pg", tag="pg")
                    for dc in range(DC):
                        nc.tensor.matmul(
                            pg[:], lhsT=w1b_sb[dc][:, ks], rhs=zo_sb[dc],
                            start=(dc == 0), stop=(dc == DC - 1),
                        )
                    nc.scalar.activation(
                        g_sb[kb][:], pg[:], AF.Identity, bias=b1_sb[kb]
                    )
                for kb in range(KB):
                    ks = slice(kb * 128, (kb + 1) * 128)
                    for th in range(2):
                        ts = slice(th * 512, (th + 1) * 512)
                        pht = ps_setup.tile([128, 512], F32, name="pht", tag="pht")
                        for dc in range(DC):
                            nc.tensor.matmul(
                                pht[:], lhsT=w1a_sb[dc][:, ks],
                                rhs=zt_sb[dc][:, ts],
                                start=(dc == 0), stop=(dc == DC - 1),
                            )
                        eng = cp_eng[kb * 2 + th]
                        if eng is nc.scalar:
                            nc.scalar.activation(htT[kb][:, ts], pht[:], AF.Copy)
                        else:
                            eng.tensor_copy(out=htT[kb][:, ts], in_=pht[:])

            # ---- main loop: 4 chunks of 16 o's ----
            ps_main_ctx = tc.psum_pool(name="ps_main", bufs=4)
            ps_main = ps_main_ctx.__enter__()
            rp = {"dve": rp_dve, "act": rp_act, "pool": rp_gps}
            pi = 0
            for ch in range(NCH):
                o0 = ch * OC
                # full-bank psum tile; only cols [0, TC*OC) used
                P = ps_main.tile([128, 512], F32, name="P", tag="P")
                nc.tensor.matmul(
                    P[:, 0:TC * OC], lhsT=zcol[:], rhs=ones128[0:1, 0:TC * OC],
                    start=True, stop=False, skip_group_check=True,
                )
                # linear terms: ct (via ztT-stationary) and co+b2 (via ones)
                for tcx in range(TC):
                    tsl = slice(tcx * 128, (tcx + 1) * 128)
                    for dc in range(DC):
                        nc.tensor.matmul(
                            P[:, tcx * OC:(tcx + 1) * OC],
                            lhsT=zt_sb[dc][:, tsl],
                            rhs=v16rep[dc][:, 0:OC],
                            start=False, stop=False, skip_group_check=True,
                        )
                        nc.tensor.matmul(
                            P[:, tcx * OC:(tcx + 1) * OC],
                            lhsT=ones128[:],
                            rhs=zom[:, dc * OL + o0:dc * OL + o0 + OC],
                            start=False, stop=False, skip_group_check=True,
                        )
                for kb in range(KB):
                    for oj in range(OC):
                        o = o0 + oj
                        e = sched[pi]
                        pi += 1
                        r = rp[e].tile([128, T], FP16, name="r", tag=f"r_{e}")
                        gcol = g_sb[kb][:, o:o + 1]
                        if e == "act":
                            nc.scalar.activation(
                                r[:], htT[kb][:], AF.Relu, bias=gcol
                            )
                        elif e == "dve":
                            nc.vector.tensor_scalar(
                                out=r[:], in0=htT[kb][:], scalar1=gcol,
                                scalar2=0.0, op0=AOP.add, op1=AOP.max,
                            )
                        else:
                            nc.gpsimd.tensor_scalar(
                                out=r[:], in0=htT[kb][:], scalar1=gcol,
                                scalar2=0.0, op0=AOP.add, op1=AOP.max,
                            )
                        for tcx in range(TC):
                            nc.tensor.matmul(
                                P[:, tcx * OC + oj:tcx * OC + oj + 1],
                                lhsT=r[:, tcx * 128:(tcx + 1) * 128],
                                rhs=w99[kb],
                                start=False, stop=(kb == KB - 1),
                                skip_group_check=True,
                            )
                # drain chunk: one DVE copy + per-t-chunk DMA
                fin = spool.tile([128, TC * OC], F32, name="fin", tag="fin")
                nc.vector.tensor_copy(out=fin[:], in_=P[:, 0:TC * OC])
                nc.sync.dma_start(
                    out=out_d[:, ch * TC * OC:(ch + 1) * TC * OC],
                    in_=fin[:],
                )
            ps_main_ctx.__exit__(None, None, None)

    nc.compile()
    return nc


def _get_nc():
    if "nc" not in _cache:
        _cache["nc"] = _build()
    return _cache["nc"]


def _host_prep(z_t, z_o, W1, b1, W2, b2):
    """Weight/layout-only host prep; returns per-core input maps."""
    zt_T = np.ascontiguousarray(z_t.T.astype(np.float16))      # [D, T]
    w1a_h = np.ascontiguousarray(W1[:D].astype(np.float16))    # [D, H]
    w1b_h = np.ascontiguousarray(W1[D:].astype(np.float16))    # [D, H]
    w99 = (0.99 * W2).astype(np.float16)                       # [H, 1]
    v = 0.01 * (W1[:D] @ W2)                                   # [D, 1]
    u = 0.01 * (W1[D:] @ W2)                                   # [D, 1]
    b2m = float(b2[0] + 0.01 * (W2[:, 0] @ b1))
    v16rep = np.repeat(v.astype(np.float16), OL, 1)            # [D, OL]
    v16rep = v16rep.reshape(DC, 128, OL).transpose(1, 0, 2).reshape(128, 2 * OL)
    w99p = w99.reshape(KB, 128).T.reshape(128, KB)             # [128, KB]
    cou_p = u.astype(np.float32).reshape(DC, 128).T.reshape(128, DC)
    b1p = b1.reshape(KB, 128).T.reshape(128, KB).astype(np.float32)
    b2qcol = np.full((128, 1), b2m / D, np.float32)
    pk32 = np.ascontiguousarray(np.concatenate([b1p, b2qcol, cou_p], 1))

    in_maps = []
    for c in range(NCORES):
        zo_T = (
            z_o[c * OL:(c + 1) * OL].T.astype(np.float16)
            .reshape(DC, 128, OL).transpose(1, 0, 2).reshape(128, 2 * OL)
        )
        pk16 = np.ascontiguousarray(
            np.concatenate([zo_T, w99p, v16rep], 1)
        )
        in_maps.append({
            "zt_T": zt_T, "w1a": w1a_h, "w1b": w1b_h,
            "pk16": pk16, "pk32": pk32,
        })
    return in_maps


def kernel(z_t, z_o, W1, b1, W2, b2, **run_kwargs):
    z_t = np.asarray(z_t, np.float32)
    z_o = np.asarray(z_o, np.float32)
    W1 = np.asarray(W1, np.float32)
    b1 = np.asarray(b1, np.float32)
    W2 = np.asarray(W2, np.float32)
    b2 = np.asarray(b2, np.float32)

    nc = _get_nc()
    in_maps = _host_prep(z_t, z_o, W1, b1, W2, b2)
    res = run_bass_kernel_spmd(
        nc, in_maps, core_ids=list(range(NCORES)), **run_kwargs
    )
    slabs = []
    for c in range(NCORES):
        a = res.results[c]["out"].reshape(128, NCH, TC, OC)
        slabs.append(a.transpose(2, 0, 1, 3).reshape(T, OL))
    out = np.concatenate(slabs, axis=1)  # [T, O]
    if run_kwargs:
        _cache["last_results"] = res
    return np.ascontiguousarray(out).astype(np.float32)


# revision 8
# speedup vs baseline: 1.0745x; 1.0306x over previous
"""Trainium2 Bass kernel for nn_CFM_80272938762374 (dense_mlp).

Reference computation (T=1024, O=512, D=256, H=512):
    ht = z_t @ W1[:D]                  # [T, H]
    ho = z_o @ W1[D:]                  # [O, H]
    h  = leaky_relu(ht[:,None,:] + ho[None,:,:] + b1, 0.01)   # [T, O, H]
    out = squeeze(h @ W2, -1) + b2[0]  # [T, O]

Strategy (8 cores, O sharded 64-wide per core; all FLOPs on device; host
does only layout prep - transposes, slicing, weight scaling/casts):

    leaky_relu(x) = 0.99*relu(x) + 0.01*x, so with g = ho + b1:
      out[t,o] = sum_k 0.99*W2[k]*relu(htT[k,t] + g[k,o])
               + ct[t] + co[o] + const        # linear terms collapse

    Per core:
    * PE computes htT[k,t] (fp16) and g[k,o] (fp32) once.
    * Main loop (64 o's x 4 k-blocks): one elementwise op produces each
      relu tile r = relu(htT + g[:,o]) [128k x 1024t] fp16; the ops are
      load-balanced across DVE (tensor_scalar, 4x mode), ACT (Relu with
      bias) and GPSIMD (tensor_scalar) so all three engines stream the
      T*O*H/8 relu volume concurrently.
    * The W2 contraction runs with r as the STATIONARY operand: per
      128-wide t-chunk, matmul(lhsT=r[:,tc], rhs=w99[kb] [128,1]) emits a
      [128t, 1] PSUM column. Columns for 16 o's x 8 t-chunks accumulate
      in one PSUM bank (bank zero-filled once via a K=1 zero matmul with
      start=True; all accumulating matmuls use start=False).
    * Linear terms enter the same PSUM bank through cheap N<=16 matmuls:
      ct via lhsT=ztT-chunk / rhs=repeat(0.01*W1a@W2), co+b2 via
      lhsT=ones / rhs=(z_oT * (0.01*W1b@W2) + b2m/256).
    * Output drains per 16-o chunk (DVE copy + DMA), overlapping the
      remaining production; final host step is a concat along O.
"""

import os

os.environ.setdefault("JAX_PLATFORMS", "axon")

import numpy as np

import concourse.bacc as bacc
import concourse.tile as tile
from concourse import mybir
from concourse.bass_utils import run_bass_kernel_spmd

F32 = mybir.dt.float32
FP16 = mybir.dt.float16
AOP = mybir.AluOpType
AF = mybir.ActivationFunctionType

T, O, D, H = 1024, 512, 256, 512
NCORES = 8
OL = O // NCORES          # 64 o's per core
KB = H // 128             # 4 k-blocks
DC = D // 128             # 2 d-chunks
TC = T // 128             # 8 t-chunks
NCH = 4                   # o-chunks per core
OC = OL // NCH            # 16 o's per chunk

_cache = {}


def _prod_schedule():
    """Static assignment of the 256 (o,kb) relu-tile productions to
    engines, greedy-balanced by modeled per-op cost, with fixed setup
    work pre-loaded per engine."""
    cost = {"dve": 327.0, "act": 1038.0, "pool": 853.0}
    load = {"dve": 3700.0, "act": 4700.0, "pool": 0.0}
    sched = []
    for _ in range(OL * KB):
        e = min(cost, key=lambda k: load[k] + cost[k])
        sched.append(e)
        load[e] += cost[e]
    for i in range(OL * KB - 4, OL * KB):
        if sched[i] != "dve":
            j = next(j for j in range(OL * KB - 16)
                     if sched[j] == "dve")
            sched[j], sched[i] = sched[i], "dve"
    return sched


def _build():
    nc = bacc.Bacc(
        "TRN2", target_bir_lowering=False, debug=False, num_devices=NCORES
    )

    zt_T = nc.dram_tensor("zt_T", [D, T], FP16, kind="ExternalInput").ap()
    w1a = nc.dram_tensor("w1a", [D, H], FP16, kind="ExternalInput").ap()
    w1b = nc.dram_tensor("w1b", [D, H], FP16, kind="ExternalInput").ap()
    # fp16 pack: zo_T (2*OL) | w99 (KB) | v16rep (2*OL)
    pk16 = nc.dram_tensor(
        "pk16", [128, 2 * OL + KB + 2 * OL], FP16, kind="ExternalInput"
    ).ap()
    # f32 pack: b1 (KB) | b2q (1) | cou (DC)
    pk32 = nc.dram_tensor(
        "pk32", [128, KB + 1 + DC], F32, kind="ExternalInput"
    ).ap()
    # per-chunk contiguous: [p, ch*128 + tc*OC + o]; host de-interleaves
    out_d = nc.dram_tensor(
        "out", [128, NCH * TC * OC], F32, kind="ExternalOutput"
    ).ap()

    sched = _prod_schedule()

    with tile.TileContext(nc) as tc:
        with (
            tc.tile_pool(name="const", bufs=1) as cpool,
            tc.tile_pool(name="rp_dve", bufs=12) as rp_dve,
            tc.tile_pool(name="rp_act", bufs=8) as rp_act,
            tc.tile_pool(name="rp_gps", bufs=8) as rp_gps,
            tc.tile_pool(name="spool", bufs=4) as spool,
        ):
            # ---- loads (ordered so htT setup can start early) ----
            zt_sb = []
            w1a_sb = []
            for dc in range(DC):
                t_ = cpool.tile([128, T], FP16, name=f"zt{dc}", tag=f"zt{dc}")
                nc.sync.dma_start(out=t_[:], in_=zt_T[dc * 128:(dc + 1) * 128, :])
                zt_sb.append(t_)
                w_ = cpool.tile([128, H], FP16, name=f"w1a{dc}", tag=f"w1a{dc}")
                nc.sync.dma_start(out=w_[:], in_=w1a[dc * 128:(dc + 1) * 128, :])
                w1a_sb.append(w_)
            w1b_sb = []
            for dc in range(DC):
                w_ = cpool.tile([128, H], FP16, name=f"w1b{dc}", tag=f"w1b{dc}")
                nc.scalar.dma_start(out=w_[:], in_=w1b[dc * 128:(dc + 1) * 128, :])
                w1b_sb.append(w_)
            p16 = cpool.tile(
                [128, 2 * OL + KB + 2 * OL], FP16, name="p16", tag="p16"
            )
            nc.scalar.dma_start(out=p16[:], in_=pk16[:])
            zo_sb = [p16[:, dc * OL:(dc + 1) * OL] for dc in range(DC)]
            w99 = [p16[:, 2 * OL + kb:2 * OL + kb + 1] for kb in range(KB)]
            v16rep = [
                p16[:, 2 * OL + KB + dc * OL:2 * OL + KB + (dc + 1) * OL]
                for dc in range(DC)
            ]
            p32 = cpool.tile([128, KB + 1 + DC], F32, name="p32", tag="p32")
            nc.scalar.dma_start(out=p32[:], in_=pk32[:])
            b1_sb = [p32[:, kb:kb + 1] for kb in range(KB)]
            b2q = p32[:, KB:KB + 1]
            cou = [p32[:, KB + 1 + dc:KB + 2 + dc] for dc in range(DC)]

            zcol = cpool.tile([1, 128], FP16, name="zcol", tag="zcol")
            nc.vector.memset(zcol[:], 0.0)
            ones128 = cpool.tile([128, 128], FP16, name="ones128", tag="ones128")
            nc.vector.memset(ones128[:], 1.0)

            # ---- setup: g then htT (PE) ----
            htT = [
                cpool.tile([128, T], FP16, name=f"htT{kb}", tag=f"htT{kb}")
                for kb in range(KB)
            ]
            g_sb = [
                cpool.tile([128, OL], F32, name=f"g{kb}", tag=f"g{kb}")
                for kb in range(KB)
            ]
            cp_eng = [nc.vector, nc.scalar, nc.vector, nc.scalar,
                      nc.vector, nc.scalar, nc.vector, nc.scalar]
            with tc.psum_pool(name="ps_setup", bufs=4) as ps_setup:
                for kb in range(KB):
                    ks = slice(kb * 128, (kb + 1) * 128)
                    pg = ps_setup.tile([128, OL], F32, name="pg", tag="pg")
                    for dc in range(DC):
                        nc.tensor.matmul(
                            pg[:], lhsT=w1b_sb[dc][:, ks], rhs=zo_sb[dc],
                            start=(dc == 0), stop=(dc == DC - 1),
                        )
                    nc.scalar.activation(
                        g_sb[kb][:], pg[:], AF.Identity, bias=b1_sb[kb]
                    )
                    for th in range(2):
                        ts = slice(th * 512, (th + 1) * 512)
                        pht = ps_setup.tile([128, 512], F32, name="pht", tag="pht")
                        for dc in range(DC):
                            nc.tensor.matmul(
                                pht[:], lhsT=w1a_sb[dc][:, ks],
                                rhs=zt_sb[dc][:, ts],
                                start=(dc == 0), stop=(dc == DC - 1),
                            )
                        eng = cp_eng[kb * 2 + th]
                        if eng is nc.scalar:
                            nc.scalar.activation(htT[kb][:, ts], pht[:], AF.Copy)
                        else:
                            eng.tensor_copy(out=htT[kb][:, ts], in_=pht[:])

            # zom[d,o] = zo_T[d,o]*cou[d] + b2m/256 (summed over d -> co+b2)
            zom = cpool.tile([128, 2 * OL], FP16, name="zom", tag="zom")
            for dc in range(DC):
                nc.gpsimd.tensor_scalar(
                    out=zom[:, dc * OL:(dc + 1) * OL], in0=zo_sb[dc],
                    scalar1=cou[dc], scalar2=b2q,
                    op0=AOP.mult, op1=AOP.add,
                )

            # ---- main loop: 4 chunks of 16 o's ----
            ps_main_ctx = tc.psum_pool(name="ps_main", bufs=4)
            ps_main = ps_main_ctx.__enter__()
            rp = {"dve": rp_dve, "act": rp_act, "pool": rp_gps}
            pi = 0
            for ch in range(NCH):
                o0 = ch * OC
                # full-bank psum tile; only cols [0, TC*OC) used
                P = ps_main.tile([128, 512], F32, name="P", tag="P")
                nc.tensor.matmul(
                    P[:, 0:TC * OC], lhsT=zcol[:], rhs=ones128[0:1, 0:TC * OC],
                    start=True, stop=False, skip_group_check=True,
                )
                # linear terms: ct (via ztT-stationary) and co+b2 (via ones)
                for tcx in range(TC):
                    tsl = slice(tcx * 128, (tcx + 1) * 128)
                    for dc in range(DC):
                        nc.tensor.matmul(
                            P[:, tcx * OC:(tcx + 1) * OC],
                            lhsT=zt_sb[dc][:, tsl],
                            rhs=v16rep[dc][:, 0:OC],
                            start=False, stop=False, skip_group_check=True,
                        )
                        nc.tensor.matmul(
                            P[:, tcx * OC:(tcx + 1) * OC],
                            lhsT=ones128[:],
                            rhs=zom[:, dc * OL + o0:dc * OL + o0 + OC],
                            start=False, stop=False, skip_group_check=True,
                        )
                for kb in range(KB):
                    for oj in range(OC):
                        o = o0 + oj
                        e = sched[pi]
                        pi += 1
                        r = rp[e].tile([128, T], FP16, name="r", tag=f"r_{e}")
                        gcol = g_sb[kb][:, o:o + 1]
                        if e == "act":
                            nc.scalar.activation(
                                r[:], htT[kb][:], AF.Relu, bias=gcol
                            )
                        elif e == "dve":
                            nc.vector.tensor_scalar(
                                out=r[:], in0=htT[kb][:], scalar1=gcol,
                                scalar2=0.0, op0=AOP.add, op1=AOP.max,
                            )
                        else:
                            nc.gpsimd.tensor_scalar(
                                out=r[:], in0=htT[kb][:], scalar1=gcol,
                                scalar2=0.0, op0=AOP.add, op1=AOP.max,
                            )
                        for tcx in range(TC):
                            nc.tensor.matmul(
                                P[:, tcx * OC + oj:tcx * OC + oj + 1],
                                lhsT=r[:, tcx * 128:(tcx + 1) * 128],
                                rhs=w99[kb],
                                start=False, stop=(kb == KB - 1),
                                skip_group_check=True,
                            )
                # drain chunk: one DVE copy + per-t-chunk DMA
                fin = spool.tile([128, TC * OC], F32, name="fin", tag="fin")
                nc.vector.tensor_copy(out=fin[:], in_=P[:, 0:TC * OC])
                nc.sync.dma_start(
                    out=out_d[:, ch * TC * OC:(ch + 1) * TC * OC],
                    in_=fin[:],
                )
            ps_main_ctx.__exit__(None, None, None)

    nc.compile()
    return nc


def _get_nc():
    if "nc" not in _cache:
        _cache["nc"] = _build()
    return _cache["nc"]


def _host_prep(z_t, z_o, W1, b1, W2, b2):
    """Weight/layout-only host prep; returns per-core input maps."""
    zt_T = np.ascontiguousarray(z_t.T.astype(np.float16))      # [D, T]
    w1a_h = np.ascontiguousarray(W1[:D].astype(np.float16))    # [D, H]
    w1b_h = np.ascontiguousarray(W1[D:].astype(np.float16))    # [D, H]
    w99 = (0.99 * W2).astype(np.float16)                       # [H, 1]
    v = 0.01 * (W1[:D] @ W2)                                   # [D, 1]
    u = 0.01 * (W1[D:] @ W2)                                   # [D, 1]
    b2m = float(b2[0] + 0.01 * (W2[:, 0] @ b1))
    v16rep = np.repeat(v.astype(np.float16), OL, 1)            # [D, OL]
    v16rep = v16rep.reshape(DC, 128, OL).transpose(1, 0, 2).reshape(128, 2 * OL)
    w99p = w99.reshape(KB, 128).T.reshape(128, KB)             # [128, KB]
    cou_p = u.astype(np.float32).reshape(DC, 128).T.reshape(128, DC)
    b1p = b1.reshape(KB, 128).T.reshape(128, KB).astype(np.float32)
    b2qcol = np.full((128, 1), b2m / D, np.float32)
    pk32 = np.ascontiguousarray(np.concatenate([b1p, b2qcol, cou_p], 1))

    in_maps = []
    for c in range(NCORES):
        zo_T = (
            z_o[c * OL:(c + 1) * OL].T.astype(np.float16)
            .reshape(DC, 128, OL).transpose(1, 0, 2).reshape(128, 2 * OL)
        )
        pk16 = np.ascontiguousarray(
            np.concatenate([zo_T, w99p, v16rep], 1)
        )
        in_maps.append({
            "zt_T": zt_T, "w1a": w1a_h, "w1b": w1b_h,
            "pk16": pk16, "pk32": pk32,
        })
    return in_maps


def kernel(z_t, z_o, W1, b1, W2, b2, **run_kwargs):
    z_t = np.asarray(z_t, np.float32)
    z_o = np.asarray(z_o, np.float32)
    W1 = np.asarray(W1, np.float32)
    b1 = np.asarray(b1, np.float32)
    W2 = np.asarray(W2, np.float32)
    b2 = np.asarray(b2, np.float32)

    nc = _get_nc()
    in_maps = _host_prep(z_t, z_o, W1, b1, W2, b2)
    res = run_bass_kernel_spmd(
        nc, in_maps, core_ids=list(range(NCORES)), **run_kwargs
    )
    slabs = []
    for c in range(NCORES):
        a = res.results[c]["out"].reshape(128, NCH, TC, OC)
        slabs.append(a.transpose(2, 0, 1, 3).reshape(T, OL))
    out = np.concatenate(slabs, axis=1)  # [T, O]
    if run_kwargs:
        _cache["last_results"] = res
    return np.ascontiguousarray(out).astype(np.float32)


# revision 9
# speedup vs baseline: 1.0806x; 1.0057x over previous
"""Trainium2 Bass kernel for nn_CFM_80272938762374 (dense_mlp).

Reference computation (T=1024, O=512, D=256, H=512):
    ht = z_t @ W1[:D]                  # [T, H]
    ho = z_o @ W1[D:]                  # [O, H]
    h  = leaky_relu(ht[:,None,:] + ho[None,:,:] + b1, 0.01)   # [T, O, H]
    out = squeeze(h @ W2, -1) + b2[0]  # [T, O]

Strategy (8 cores, O sharded 64-wide per core; all FLOPs on device; host
does only layout prep - transposes, slicing, weight scaling/casts):

    leaky_relu(x) = 0.99*relu(x) + 0.01*x, so with g = ho + b1:
      out[t,o] = sum_k 0.99*W2[k]*relu(htT[k,t] + g[k,o])
               + ct[t] + co[o] + const        # linear terms collapse

    Per core:
    * PE computes htT[k,t] (fp16) and g[k,o] (fp32) once.
    * Main loop (64 o's x 4 k-blocks): one elementwise op produces each
      relu tile r = relu(htT + g[:,o]) [128k x 1024t] fp16; the ops are
      load-balanced across DVE (tensor_scalar, 4x mode), ACT (Relu with
      bias) and GPSIMD (tensor_scalar) so all three engines stream the
      T*O*H/8 relu volume concurrently.
    * The W2 contraction runs with r as the STATIONARY operand: per
      128-wide t-chunk, matmul(lhsT=r[:,tc], rhs=w99[kb] [128,1]) emits a
      [128t, 1] PSUM column. Columns for 16 o's x 8 t-chunks accumulate
      in one PSUM bank (bank zero-filled once via a K=1 zero matmul with
      start=True; all accumulating matmuls use start=False).
    * Linear terms enter the same PSUM bank through cheap N<=16 matmuls:
      ct via lhsT=ztT-chunk / rhs=repeat(0.01*W1a@W2), co+b2 via
      lhsT=ones / rhs=(z_oT * (0.01*W1b@W2) + b2m/256).
    * Output drains per 16-o chunk (DVE copy + DMA), overlapping the
      remaining production; final host step is a concat along O.
"""

import os

os.environ.setdefault("JAX_PLATFORMS", "axon")

import numpy as np

import concourse.bacc as bacc
import concourse.tile as tile
from concourse import mybir
from concourse.bass_utils import run_bass_kernel_spmd

F32 = mybir.dt.float32
FP16 = mybir.dt.float16
AOP = mybir.AluOpType
AF = mybir.ActivationFunctionType

T, O, D, H = 1024, 512, 256, 512
NCORES = 8
OL = O // NCORES          # 64 o's per core
KB = H // 128             # 4 k-blocks
DC = D // 128             # 2 d-chunks
TC = T // 128             # 8 t-chunks
NCH = 4                   # o-chunks per core
OC = OL // NCH            # 16 o's per chunk

_cache = {}


def _prod_schedule():
    """Static assignment of the 256 (o,kb) relu-tile productions to
    engines, greedy-balanced by modeled per-op cost, with fixed setup
    work pre-loaded per engine."""
    cost = {"dve": 327.0, "act": 1038.0, "pool": 853.0}
    load = {"dve": 3700.0, "act": 3800.0, "pool": 900.0}
    sched = []
    for _ in range(OL * KB):
        e = min(cost, key=lambda k: load[k] + cost[k])
        sched.append(e)
        load[e] += cost[e]
    for i in range(OL * KB - 4, OL * KB):
        if sched[i] != "dve":
            j = next(j for j in range(OL * KB - 16)
                     if sched[j] == "dve")
            sched[j], sched[i] = sched[i], "dve"
    return sched


def _build():
    nc = bacc.Bacc(
        "TRN2", target_bir_lowering=False, debug=False, num_devices=NCORES
    )

    zt_T = nc.dram_tensor("zt_T", [D, T], FP16, kind="ExternalInput").ap()
    w1a = nc.dram_tensor("w1a", [D, H], FP16, kind="ExternalInput").ap()
    w1b = nc.dram_tensor("w1b", [D, H], FP16, kind="ExternalInput").ap()
    # fp16 pack: zo_T (2*OL) | w99 (KB) | v16rep (2*OL)
    pk16 = nc.dram_tensor(
        "pk16", [128, 2 * OL + KB + 2 * OL], FP16, kind="ExternalInput"
    ).ap()
    # f32 pack: b1 (KB) | b2q (1) | cou (DC)
    pk32 = nc.dram_tensor(
        "pk32", [128, KB + 1 + DC], F32, kind="ExternalInput"
    ).ap()
    # per-chunk contiguous: [p, ch*128 + tc*OC + o]; host de-interleaves
    out_d = nc.dram_tensor(
        "out", [128, NCH * TC * OC], F32, kind="ExternalOutput"
    ).ap()

    sched = _prod_schedule()

    with tile.TileContext(nc) as tc:
        with (
            tc.tile_pool(name="const", bufs=1) as cpool,
            tc.tile_pool(name="rp_dve", bufs=12) as rp_dve,
            tc.tile_pool(name="rp_act", bufs=8) as rp_act,
            tc.tile_pool(name="rp_gps", bufs=8) as rp_gps,
            tc.tile_pool(name="spool", bufs=4) as spool,
        ):
            # ---- loads (ordered so htT setup can start early) ----
            zt_sb = []
            w1a_sb = []
            for dc in range(DC):
                t_ = cpool.tile([128, T], FP16, name=f"zt{dc}", tag=f"zt{dc}")
                nc.sync.dma_start(
                    out=t_[:, 0:512], in_=zt_T[dc * 128:(dc + 1) * 128, 0:512]
                )
                zt_sb.append(t_)
                w_ = cpool.tile([128, H], FP16, name=f"w1a{dc}", tag=f"w1a{dc}")
                nc.sync.dma_start(out=w_[:], in_=w1a[dc * 128:(dc + 1) * 128, :])
                w1a_sb.append(w_)
            for dc in range(DC):
                nc.sync.dma_start(
                    out=zt_sb[dc][:, 512:1024],
                    in_=zt_T[dc * 128:(dc + 1) * 128, 512:1024],
                )
            w1b_sb = []
            for dc in range(DC):
                w_ = cpool.tile([128, H], FP16, name=f"w1b{dc}", tag=f"w1b{dc}")
                nc.scalar.dma_start(out=w_[:], in_=w1b[dc * 128:(dc + 1) * 128, :])
                w1b_sb.append(w_)
            p16 = cpool.tile(
                [128, 2 * OL + KB + 2 * OL], FP16, name="p16", tag="p16"
            )
            nc.scalar.dma_start(out=p16[:], in_=pk16[:])
            zo_sb = [p16[:, dc * OL:(dc + 1) * OL] for dc in range(DC)]
            w99 = [p16[:, 2 * OL + kb:2 * OL + kb + 1] for kb in range(KB)]
            v16rep = [
                p16[:, 2 * OL + KB + dc * OL:2 * OL + KB + (dc + 1) * OL]
                for dc in range(DC)
            ]
            p32 = cpool.tile([128, KB + 1 + DC], F32, name="p32", tag="p32")
            nc.scalar.dma_start(out=p32[:], in_=pk32[:])
            b1_sb = [p32[:, kb:kb + 1] for kb in range(KB)]
            b2q = p32[:, KB:KB + 1]
            cou = [p32[:, KB + 1 + dc:KB + 2 + dc] for dc in range(DC)]

            zcol = cpool.tile([1, 128], FP16, name="zcol", tag="zcol")
            nc.vector.memset(zcol[:], 0.0)
            ones128 = cpool.tile([128, 128], FP16, name="ones128", tag="ones128")
            nc.vector.memset(ones128[:], 1.0)

            # ---- setup: g then htT (PE) ----
            htT = [
                cpool.tile([128, T], FP16, name=f"htT{kb}", tag=f"htT{kb}")
                for kb in range(KB)
            ]
            g_sb = [
                cpool.tile([128, OL], F32, name=f"g{kb}", tag=f"g{kb}")
                for kb in range(KB)
            ]
            cp_eng = [nc.vector, nc.scalar, nc.vector, nc.scalar,
                      nc.vector, nc.scalar, nc.vector, nc.scalar]
            with tc.psum_pool(name="ps_setup", bufs=4) as ps_setup:
                for kb in range(KB):
                    ks = slice(kb * 128, (kb + 1) * 128)
                    pg = ps_setup.tile([128, OL], F32, name="pg", tag="pg")
                    for dc in range(DC):
                        nc.tensor.matmul(
                            pg[:], lhsT=w1b_sb[dc][:, ks], rhs=zo_sb[dc],
                            start=(dc == 0), stop=(dc == DC - 1),
                        )
                    nc.scalar.activation(
                        g_sb[kb][:], pg[:], AF.Identity, bias=b1_sb[kb]
                    )
                    for th in range(2):
                        ts = slice(th * 512, (th + 1) * 512)
                        pht = ps_setup.tile([128, 512], F32, name="pht", tag="pht")
                        for dc in range(DC):
                            nc.tensor.matmul(
                                pht[:], lhsT=w1a_sb[dc][:, ks],
                                rhs=zt_sb[dc][:, ts],
                                start=(dc == 0), stop=(dc == DC - 1),
                            )
                        eng = cp_eng[kb * 2 + th]
                        if eng is nc.scalar:
                            nc.scalar.activation(htT[kb][:, ts], pht[:], AF.Copy)
                        else:
                            eng.tensor_copy(out=htT[kb][:, ts], in_=pht[:])

            # zom[d,o] = zo_T[d,o]*cou[d] + b2m/256 (summed over d -> co+b2)
            zom = cpool.tile([128, 2 * OL], FP16, name="zom", tag="zom")
            for dc in range(DC):
                nc.gpsimd.tensor_scalar(
                    out=zom[:, dc * OL:(dc + 1) * OL], in0=zo_sb[dc],
                    scalar1=cou[dc], scalar2=b2q,
                    op0=AOP.mult, op1=AOP.add,
                )

            # ---- main loop: 4 chunks of 16 o's ----
            ps_main_ctx = tc.psum_pool(name="ps_main", bufs=4)
            ps_main = ps_main_ctx.__enter__()
            rp = {"dve": rp_dve, "act": rp_act, "pool": rp_gps}
            pi = 0
            for ch in range(NCH):
                o0 = ch * OC
                # full-bank psum tile; only cols [0, TC*OC) used
                P = ps_main.tile([128, 512], F32, name="P", tag="P")
                nc.tensor.matmul(
                    P[:, 0:TC * OC], lhsT=zcol[:], rhs=ones128[0:1, 0:TC * OC],
                    start=True, stop=False, skip_group_check=True,
                )
                # linear terms: ct (via ztT-stationary) and co+b2 (via ones)
                for tcx in range(TC):
                    tsl = slice(tcx * 128, (tcx + 1) * 128)
                    for dc in range(DC):
                        nc.tensor.matmul(
                            P[:, tcx * OC:(tcx + 1) * OC],
                            lhsT=zt_sb[dc][:, tsl],
                            rhs=v16rep[dc][:, 0:OC],
                            start=False, stop=False, skip_group_check=True,
                        )
                        nc.tensor.matmul(
                            P[:, tcx * OC:(tcx + 1) * OC],
                            lhsT=ones128[:],
                            rhs=zom[:, dc * OL + o0:dc * OL + o0 + OC],
                            start=False, stop=False, skip_group_check=True,
                        )
                half_done = {}
                for kb in range(KB):
                    for oj in range(OC):
                        o = o0 + oj
                        e = sched[pi]
                        pi += 1
                        r = rp[e].tile([128, T], FP16, name="r", tag=f"r_{e}")
                        gcol = g_sb[kb][:, o:o + 1]
                        if e == "act":
                            nc.scalar.activation(
                                r[:], htT[kb][:], AF.Relu, bias=gcol
                            )
                        elif e == "dve":
                            nc.vector.tensor_scalar(
                                out=r[:], in0=htT[kb][:], scalar1=gcol,
                                scalar2=0.0, op0=AOP.add, op1=AOP.max,
                            )
                        else:
                            nc.gpsimd.tensor_scalar(
                                out=r[:], in0=htT[kb][:], scalar1=gcol,
                                scalar2=0.0, op0=AOP.add, op1=AOP.max,
                            )
                        for tcx in range(TC):
                            nc.tensor.matmul(
                                P[:, tcx * OC + oj:tcx * OC + oj + 1],
                                lhsT=r[:, tcx * 128:(tcx + 1) * 128],
                                rhs=w99[kb],
                                start=False, stop=(kb == KB - 1),
                                skip_group_check=True,
                            )
                # drain chunk in two o-halves (psum cols are tc-major, so
                # each half is a strided read of OC/2 cols per t-chunk)
                for hf in range(2):
                    cs = hf * (OC // 2)
                    fin = spool.tile(
                        [128, TC * OC // 2], F32, name="fin", tag="fin"
                    )
                    nc.vector.tensor_copy(
                        out=fin[:],
                        in_=P[:, 0:TC * OC].rearrange(
                            "p (t o) -> p t o", o=OC
                        )[:, :, cs:cs + OC // 2],
                    )
                    nc.sync.dma_start(
                        out=out_d[:, ch * TC * OC + hf * TC * OC // 2:
                                  ch * TC * OC + (hf + 1) * TC * OC // 2],
                        in_=fin[:],
                    )
            ps_main_ctx.__exit__(None, None, None)

    nc.compile()
    return nc


def _get_nc():
    if "nc" not in _cache:
        _cache["nc"] = _build()
    return _cache["nc"]


def _host_prep(z_t, z_o, W1, b1, W2, b2):
    """Weight/layout-only host prep; returns per-core input maps."""
    zt_T = np.ascontiguousarray(z_t.T.astype(np.float16))      # [D, T]
    w1a_h = np.ascontiguousarray(W1[:D].astype(np.float16))    # [D, H]
    w1b_h = np.ascontiguousarray(W1[D:].astype(np.float16))    # [D, H]
    w99 = (0.99 * W2).astype(np.float16)                       # [H, 1]
    v = 0.01 * (W1[:D] @ W2)                                   # [D, 1]
    u = 0.01 * (W1[D:] @ W2)                                   # [D, 1]
    b2m = float(b2[0] + 0.01 * (W2[:, 0] @ b1))
    v16rep = np.repeat(v.astype(np.float16), OL, 1)            # [D, OL]
    v16rep = v16rep.reshape(DC, 128, OL).transpose(1, 0, 2).reshape(128, 2 * OL)
    w99p = w99.reshape(KB, 128).T.reshape(128, KB)             # [128, KB]
    cou_p = u.astype(np.float32).reshape(DC, 128).T.reshape(128, DC)
    b1p = b1.reshape(KB, 128).T.reshape(128, KB).astype(np.float32)
    b2qcol = np.full((128, 1), b2m / D, np.float32)
    pk32 = np.ascontiguousarray(np.concatenate([b1p, b2qcol, cou_p], 1))

    in_maps = []
    for c in range(NCORES):
        zo_T = (
            z_o[c * OL:(c + 1) * OL].T.astype(np.float16)
            .reshape(DC, 128, OL).transpose(1, 0, 2).reshape(128, 2 * OL)
        )
        pk16 = np.ascontiguousarray(
            np.concatenate([zo_T, w99p, v16rep], 1)
        )
        in_maps.append({
            "zt_T": zt_T, "w1a": w1a_h, "w1b": w1b_h,
            "pk16": pk16, "pk32": pk32,
        })
    return in_maps


def kernel(z_t, z_o, W1, b1, W2, b2, **run_kwargs):
    z_t = np.asarray(z_t, np.float32)
    z_o = np.asarray(z_o, np.float32)
    W1 = np.asarray(W1, np.float32)
    b1 = np.asarray(b1, np.float32)
    W2 = np.asarray(W2, np.float32)
    b2 = np.asarray(b2, np.float32)

    nc = _get_nc()
    in_maps = _host_prep(z_t, z_o, W1, b1, W2, b2)
    res = run_bass_kernel_spmd(
        nc, in_maps, core_ids=list(range(NCORES)), **run_kwargs
    )
    slabs = []
    for c in range(NCORES):
        a = res.results[c]["out"].reshape(128, NCH, 2, TC, OC // 2)
        # [p, ch, hf, tc, oj] -> [tc, p, ch, hf, oj] -> [T, OL]
        slabs.append(a.transpose(3, 0, 1, 2, 4).reshape(T, OL))
    out = np.concatenate(slabs, axis=1)  # [T, O]
    if run_kwargs:
        _cache["last_results"] = res
    return np.ascontiguousarray(out).astype(np.float32)
